# revision 16
# baseline (speedup 1.0000x reference)
"""Trainium2 Bass kernel for nn_CoreDiffusion (gnn_message_passing).

Sharding: node dim N=4096 split across 8 cores (512 nodes each).

Host-side staging (per core k):
  - adj is quantized to fp8 e3m4 after centering to [-0.5, 0.5) with
    error-feedback across the c (core-diffusion) dim: q_c = Q(A_c - 0.5 + carry),
    carry' = (A_c - 0.5 + carry) - q_c.  The cumsum over c then telescopes so
    quantization error does not accumulate across GRU steps.  The removed 0.5
    mean is restored on device as a rank-1 bias 0.5*(c+1)*colsum(x16)[d]
    (shipped as the tiny `sbias` input — standard zero-point correction).
  - adj rows for core k are pre-transposed to [B, C, 128(p), JC, NS] so the
    contraction dim j = jc*128+p lands on SBUF partitions with 4KB contiguous
    DMA runs; no on-chip transposes are needed.
  - x is pre-cast to fp16 and packed to the SBUF layout [128, JC, B, D].

Device (per core), software-pipelined over (c, b):
  - Phase A: msgT[d,i] accumulates adj-chunks (fp8e3 moving) against x16
    (fp16 stationary) directly into a per-b persistent PSUM bank, so the
    cumsum over c happens inside PSUM accumulation for free.
  - hx_c = relu(psum + sbias) fused on DVE (tensor_scalar add+max).
  - GRU step (one slot behind Phase A so PE never stalls), in two NS/2
    halves so each gate-pair PSUM tile is a single bank (one start=True per
    2KB zero region).  h_t carries a ones-row so the hidden-side gate
    biases ride the matmuls; x-side biases ride the Act activations.  h_t
    starts zeroed so c=0 needs no special casing.  Pointwise is emitted
    stage-by-stage across the halves so the in-order DVE/Act queues stream.
  - The GRU-output sum over c is formed as PSUM-accumulated PE transposes
    of h_t (transpose is linear), two slots behind Phase A, so LayerNorm
    reads a ready [node, feat] tile at the end with no serial reduction.
  - LayerNorm: bn_stats/aggr + fused (x-mu)*rstd via two-op tensor_scalar.
No collectives; full output gathered on host.
"""
import numpy as np
import ml_dtypes
from contextlib import ExitStack

import concourse.bass as bass
import concourse.mybir as mybir
import concourse.tile as tile
from concourse import bacc
from concourse.masks import make_identity
from concourse.bass_utils import run_bass_kernel_spmd

F32 = mybir.dt.float32
F32R = mybir.dt.float32r
F16 = mybir.dt.float16
F8E3 = mybir.dt.float8e3
AF = mybir.ActivationFunctionType
ALU = mybir.AluOpType

B, C, N, D, H = 2, 4, 4096, 64, 64
NCORES = 8
NS = N // NCORES            # 512 nodes per core
HNS = NS // 2               # GRU half-tile
JC = N // 128               # 32 contraction chunks
NJ = 4                      # adj DMA chunks per (b, c)
JCD = JC // NJ              # 8 contraction chunks per DMA
LN_EPS = 1e-5
NBLK = B * NS // 128        # 8 output row-blocks of 128


def build():
    nc = bacc.Bacc("TRN2", target_bir_lowering=False, debug=False,
                   num_devices=NCORES)
    adj_t = nc.declare_dram_parameter("adj_t", [B, C, 128, JC, NS], F8E3,
                                      isOutput=False)
    x_p = nc.declare_dram_parameter("x_p", [128, JC, B, D], F16, isOutput=False)
    sb_in = nc.declare_dram_parameter("sb_in", [64, B, C], F32, isOutput=False)
    w_ih = nc.declare_dram_parameter("w_ih", [3 * H, D], F32, isOutput=False)
    w_hh = nc.declare_dram_parameter("w_hh", [3 * H, H], F32, isOutput=False)
    b_ih = nc.declare_dram_parameter("b_ih", [3 * H], F32, isOutput=False)
    b_hh = nc.declare_dram_parameter("b_hh", [3 * H], F32, isOutput=False)
    gamma = nc.declare_dram_parameter("gamma", [H], F32, isOutput=False)
    beta = nc.declare_dram_parameter("beta", [H], F32, isOutput=False)
    out_s = nc.declare_dram_parameter("out_s", [B, NS, H], F32, isOutput=True)

    with tile.TileContext(nc) as tc, ExitStack() as ctx:
        const = ctx.enter_context(tc.tile_pool(name="const", bufs=1))
        adj_pool = ctx.enter_context(tc.tile_pool(name="adj", bufs=8))
        gru = ctx.enter_context(tc.tile_pool(name="gru", bufs=2))
        psum_acc = ctx.enter_context(tc.tile_pool(name="psA", bufs=1, space="PSUM"))
        psum_g = ctx.enter_context(tc.tile_pool(name="psG", bufs=2, space="PSUM"))
        psum_o = ctx.enter_context(tc.tile_pool(name="psO", bufs=1, space="PSUM"))
        psum_m = ctx.enter_context(tc.tile_pool(name="psM", bufs=1, space="PSUM"))

        # ---------- setup (cheap, non-blocking) ----------
        ident = const.tile([128, 128], F32)
        make_identity(nc, ident)
        ident_r = const.tile([64, 64], F32R)
        nc.vector.tensor_copy(ident_r, ident[0:64, 0:64])
        eps_sb = const.tile([128, 1], F32)
        nc.vector.memset(eps_sb, LN_EPS)

        # x16 quarters + first adj half-chunks interleaved on the sync queue
        # so the first Phase-A matmul can start ~4us in.
        x16 = const.tile([128, JC, B, D], F16)
        a_first = []
        for q in range(4):
            qs = JC // 4
            nc.sync.dma_start(x16[:, q * qs:(q + 1) * qs, :, :],
                              x_p[:, q * qs:(q + 1) * qs, :, :])
            for hh in range(2):
                a_in = adj_pool.tile([128, JCD // 2, NS], F8E3, tag="a_first")
                j0 = q * JCD + hh * (JCD // 2)
                nc.sync.dma_start(a_in, adj_t[0, 0, :, j0:j0 + JCD // 2, :])
                a_first.append(a_in)
        sbias = const.tile([64, B, C], F32)
        nc.sync.dma_start(sbias, sb_in[:, :, :])
        wih_sb = const.tile([128, 2, D], F32)
        nc.sync.dma_start(wih_sb[:, 0, :], w_ih[0:128, :])
        nc.sync.dma_start(wih_sb[0:64, 1, :], w_ih[128:192, :])
        whh_sb = const.tile([128, 2, H], F32)
        nc.sync.dma_start(whh_sb[:, 0, :], w_hh[0:128, :])
        nc.sync.dma_start(whh_sb[0:64, 1, :], w_hh[128:192, :])

        # x-side gate biases ride the activations: bsum = b_ih + b_hh (r, z),
        # b_ih_n on the tanh.  b_hh_n rides wg[5]'s ones-row instead.
        bsum = const.tile([64, 3], F32)
        bih_sb = const.tile([64, 3], F32)
        nc.sync.dma_start(bih_sb, b_ih.rearrange("(g p) -> p g", p=64))
        bhh_sb = const.tile([64, 3], F32)
        nc.sync.dma_start(bhh_sb, b_hh.rearrange("(g p) -> p g", p=64))
        nc.vector.tensor_add(bsum, bih_sb, bhh_sb)

        gam_sb = const.tile([128, H], F32)
        g_ap = gamma[:]
        nc.gpsimd.dma_start(out=gam_sb, in_=bass.AP(
            tensor=g_ap.tensor, offset=g_ap.offset, ap=[[0, 128]] + list(g_ap.ap)))
        bet_sb = const.tile([128, H], F32)
        b_ap = beta[:]
        nc.gpsimd.dma_start(out=bet_sb, in_=bass.AP(
            tensor=b_ap.tensor, offset=b_ap.offset, ap=[[0, 128]] + list(b_ap.ap)))

        # persistent state; h_t row 64 is the ones-row feeding wg[3..5] bias
        # rows, rows 0:64 start at zero (c=0 hidden side contributes b_hh).
        hx = const.tile([64, C, B * NS], F32R)
        h_t = const.tile([65, B * NS], F32R)
        stage1 = const.tile([1, B * NS], F32)
        nc.vector.memset(stage1, 1.0)
        nc.vector.tensor_copy(h_t[64:65, :], stage1)
        stage0 = const.tile([64, B * NS], F32)
        nc.vector.memset(stage0, 0.0)
        nc.vector.tensor_copy(h_t[0:64, :], stage0)

        ps_acc = psum_acc.tile([64, B, NS], F32)    # per-b running cumsum
        ps_oT = psum_o.tile([128, NBLK, H], F32R)   # sum_c h_c^T, one bank
        # rx, zx, nx: [64, 64]; rh, zh, nh: [65, 64] with bias row
        # (0, 0, b_hh_n)
        wg = [const.tile([65 if i >= 3 else 64, 64], F32R, name=f"wg{i}")
              for i in range(6)]

        def emit_phase_a(c, b, chunks=None):
            if chunks is not None:
                for jl in range(JC):
                    a_in = chunks[jl // (JCD // 2)]
                    nc.tensor.matmul(
                        ps_acc[:, b, :], x16[:, jl, b, :],
                        a_in[:, jl % (JCD // 2), :],
                        start=(c == 0 and jl == 0), stop=(jl == JC - 1),
                        skip_group_check=True)
            else:
                for jd in range(NJ):
                    a_in = adj_pool.tile([128, JCD, NS], F8E3, tag="a_in")
                    nc.sync.dma_start(
                        a_in, adj_t[b, c, :, jd * JCD:(jd + 1) * JCD, :])
                    for jl in range(JCD):
                        jc = jd * JCD + jl
                        nc.tensor.matmul(
                            ps_acc[:, b, :], x16[:, jc, b, :], a_in[:, jl, :],
                            start=(c == 0 and jc == 0), stop=(jc == JC - 1),
                            skip_group_check=True)
            # hx_c = relu(cumsum + 0.5*(c+1)*colsum(x)) fused on DVE
            nc.vector.tensor_scalar(
                out=hx[:, c, b * NS:(b + 1) * NS], in0=ps_acc[:, b, :],
                scalar1=sbias[:, b, c:c + 1], scalar2=0.0,
                op0=ALU.add, op1=ALU.max)

        def gru_mm(c, b):
            halves = []
            for hf in range(2):
                sl = slice(b * NS + hf * HNS, b * NS + (hf + 1) * HNS)
                hx_c = hx[:, c, sl]
                h_c = h_t[:, sl]
                # one start=True per 2KB zero region; siblings rely on
                # first-touch zeroing
                ps_rz = psum_g.tile([64, 2, HNS], F32, tag="rz")
                nc.tensor.matmul(ps_rz[:, 0, :], wg[0], hx_c,
                                 start=True, stop=False)
                nc.tensor.matmul(ps_rz[:, 1, :], wg[1], hx_c,
                                 start=False, stop=False)
                nc.tensor.matmul(ps_rz[:, 0, :], wg[3], h_c,
                                 start=False, stop=False)
                nc.tensor.matmul(ps_rz[:, 1, :], wg[4], h_c,
                                 start=False, stop=True)
                ps_nh = psum_g.tile([64, 2, HNS], F32, tag="nh")
                nc.tensor.matmul(ps_nh[:, 0, :], wg[2], hx_c,
                                 start=True, stop=False)
                nc.tensor.matmul(ps_nh[:, 1, :], wg[5], h_c,
                                 start=False, stop=True)
                halves.append((ps_rz, ps_nh, sl))
            return halves

        def gru_pw(c, b, halves):
            # stage-by-stage across the two halves so the in-order DVE/Act
            # queues stream instead of serializing the dependency chains
            rzs, t1s, t2s, nss, t3s, t4s = [], [], [], [], [], []
            for i, (ps_rz, ps_nh, sl) in enumerate(halves):
                rz = gru.tile([64, 2, HNS], F32, tag="rz", name=f"rz{i}")
                nc.scalar.activation(rz[:, 0, :], ps_rz[:, 0, :], AF.Sigmoid,
                                     bias=bsum[:, 0:1])
                nc.scalar.activation(rz[:, 1, :], ps_rz[:, 1, :], AF.Sigmoid,
                                     bias=bsum[:, 1:2])
                rzs.append(rz)
            for i, (ps_rz, ps_nh, sl) in enumerate(halves):
                t1 = gru.tile([64, HNS], F32, tag="t1", name=f"t1_{i}")
                nc.vector.tensor_mul(t1, rzs[i][:, 0, :], ps_nh[:, 1, :])
                t1s.append(t1)
            for i, (ps_rz, ps_nh, sl) in enumerate(halves):
                t2 = gru.tile([64, HNS], F32, tag="t2", name=f"t2_{i}")
                nc.vector.tensor_add(t2, t1s[i], ps_nh[:, 0, :])
                t2s.append(t2)
            for i in range(2):
                n_sb = gru.tile([64, HNS], F32, tag="n", name=f"n{i}")
                nc.scalar.activation(n_sb, t2s[i], AF.Tanh, bias=bih_sb[:, 2:3])
                nss.append(n_sb)
            for i, (ps_rz, ps_nh, sl) in enumerate(halves):
                t3 = gru.tile([64, HNS], F32, tag="t3", name=f"t3_{i}")
                nc.vector.tensor_sub(t3, h_t[0:64, sl], nss[i])
                t3s.append(t3)
            for i in range(2):
                t4 = gru.tile([64, HNS], F32, tag="t4", name=f"t4_{i}")
                nc.vector.tensor_mul(t4, rzs[i][:, 1, :], t3s[i])
                t4s.append(t4)
            for i, (ps_rz, ps_nh, sl) in enumerate(halves):
                nc.vector.tensor_add(h_t[0:64, sl], nss[i], t4s[i])

        def emit_gru(c, b):
            gru_pw(c, b, gru_mm(c, b))

        def emit_hT(c, b):
            # accumulate h_c^T into ps_oT (transpose is linear).  All NBLK
            # regions live in one bank: a single start=True arms the whole
            # zero region; every other write relies on first-touch zeroing.
            for blk in range(NS // 128):
                gblk = b * (NS // 128) + blk
                nc.tensor.matmul(
                    ps_oT[:, gblk, :], h_t[0:64, bass.ts(b * (NS // 128) + blk, 128)],
                    ident_r,
                    start=(c == 0 and gblk == 0),
                    stop=(c == C - 1 and gblk == NBLK - 1),
                    is_transpose=True, skip_group_check=True)

        # LayerNorm over a [128, H] row-block
        stats = const.tile([128, NBLK, 6], F32)
        mv = const.tile([128, NBLK, 2], F32)
        rstd = const.tile([128, NBLK, 1], F32)
        out_st = const.tile([128, NBLK, H], F32)

        def emit_ln(b):
            for blk in range(b * (NS // 128), (b + 1) * (NS // 128)):
                nc.vector.bn_stats(stats[:, blk, :], ps_oT[:, blk, :])
                nc.vector.bn_aggr(mv[:, blk, :], stats[:, blk, :])
            for blk in range(b * (NS // 128), (b + 1) * (NS // 128)):
                nc.scalar.activation(rstd[:, blk, :], mv[:, blk, 1:2],
                                     AF.Sqrt, bias=eps_sb)
            for blk in range(b * (NS // 128), (b + 1) * (NS // 128)):
                nc.vector.reciprocal(rstd[:, blk, :], rstd[:, blk, :])
            for blk in range(b * (NS // 128), (b + 1) * (NS // 128)):
                xm = gru.tile([128, H], F32, tag="xm")
                nc.vector.tensor_scalar(
                    out=xm, in0=ps_oT[:, blk, :], scalar1=mv[:, blk, 0:1],
                    scalar2=rstd[:, blk, 0:1], op0=ALU.subtract, op1=ALU.mult)
                nc.vector.tensor_mul(xm, xm, gam_sb)
                nc.vector.tensor_add(out_st[:, blk, :], xm, bet_sb)
            nc.sync.dma_start(
                out_s[b].rearrange("(q p) d -> p q d", p=128),
                out_st[:, b * (NS // 128):(b + 1) * (NS // 128), :])

        # ---------- main loop ----------
        emit_phase_a(0, 0, chunks=a_first)
        # GRU weight stationaries: transpose after slot 0's matmuls so they
        # do not delay the first Phase-A work on PE.
        for gi, (src, blk, prow, tag) in enumerate([
                (wih_sb, 0, 0, "rz"), (wih_sb, 0, 64, "nh"), (wih_sb, 1, 0, "m"),
                (whh_sb, 0, 0, "rz"), (whh_sb, 0, 64, "nh"), (whh_sb, 1, 0, "m")]):
            pool_w = psum_m if tag == "m" else psum_g
            ps_w = pool_w.tile([64, 64], F32, tag=tag, name=f"psw{gi}")
            nc.tensor.transpose(ps_w, src[prow:prow + 64, blk, :],
                                ident[prow:prow + 64, prow:prow + 64])
            nc.vector.tensor_copy(wg[gi][0:64, :], ps_w)
        # bias rows for the hidden-side stationaries: (0, 0, b_hh_n)
        zrow = const.tile([1, 64], F32)
        nc.vector.memset(zrow, 0.0)
        nc.vector.tensor_copy(wg[3][64:65, :], zrow)
        nc.vector.tensor_copy(wg[4][64:65, :], zrow)
        brow = const.tile([1, 64], F32)
        nc.gpsimd.dma_start(out=brow,
                            in_=b_hh[128:192].rearrange("(a f) -> a f", a=1))
        nc.vector.tensor_copy(wg[5][64:65, :], brow)

        pend1 = (0, 0)      # awaiting GRU
        pend2 = None        # awaiting h^T accumulation
        for c in range(C):
            for b in range(B):
                if (c, b) == (0, 0):
                    continue
                emit_phase_a(c, b)
                if pend2 is not None:
                    emit_hT(*pend2)
                emit_gru(*pend1)
                pend2 = pend1
                pend1 = (c, b)
        # tail: G(3,0) transposes, then last GRU overlapped with b=0 LN
        emit_hT(*pend2)
        halves = gru_mm(*pend1)
        gru_pw(*pend1, halves)
        emit_hT(*pend1)
        emit_ln(0)
        emit_ln(1)

    nc.compile()
    return nc


_NC_CACHE = None


def _get_nc():
    global _NC_CACHE
    if _NC_CACHE is None:
        _NC_CACHE = build()
    return _NC_CACHE


def _quantize_adj(adj):
    """Center to [-0.5, 0.5), quantize to fp8 e3m4 with error feedback
    across the c dim so the on-device cumsum telescopes the error."""
    q = np.empty((B, C, N, N), dtype=ml_dtypes.float8_e3m4)
    carry = np.zeros((B, N, N), np.float32)
    for c in range(C):
        t = (adj[:, c] - 0.5) + carry
        qc = t.astype(ml_dtypes.float8_e3m4)
        carry = t - qc.astype(np.float32)
        q[:, c] = qc
    return q


def run(inputs, **spmd_kwargs):
    nc = _get_nc()
    adj = np.asarray(inputs["adj"], dtype=np.float32)
    adj8 = _quantize_adj(adj)
    xq = np.asarray(inputs["x"], dtype=np.float32).astype(np.float16)
    # [128(p), JC, B, D] with j = jc*128 + p
    x_p = np.ascontiguousarray(xq.reshape(B, JC, 128, D).transpose(2, 1, 0, 3))
    # zero-point correction: sbias[d, b, c] = 0.5*(c+1)*sum_j x16[b, j, d]
    colsum = xq.astype(np.float32).sum(axis=1)            # [B, D]
    sb = np.ascontiguousarray(
        np.einsum('bd,c->dbc', colsum, 0.5 * np.arange(1, C + 1,
                                                       dtype=np.float32)))
    in_maps = []
    for k in range(NCORES):
        # [B, C, 128(p), JC, NS] with j = jc*128 + p, i local to core k
        a_k = adj8[:, :, k * NS:(k + 1) * NS, :]              # [B, C, NS(i), N(j)]
        a_k = a_k.reshape(B, C, NS, JC, 128).transpose(0, 1, 4, 3, 2)
        m = {
            "adj_t": np.ascontiguousarray(a_k),
            "x_p": x_p,
            "sb_in": sb,
            "w_ih": np.ascontiguousarray(inputs["w_ih"], dtype=np.float32),
            "w_hh": np.ascontiguousarray(inputs["w_hh"], dtype=np.float32),
            "b_ih": np.ascontiguousarray(inputs["b_ih"], dtype=np.float32),
            "b_hh": np.ascontiguousarray(inputs["b_hh"], dtype=np.float32),
            "gamma": np.ascontiguousarray(inputs["gamma"], dtype=np.float32),
            "beta": np.ascontiguousarray(inputs["beta"], dtype=np.float32),
        }
        in_maps.append(m)
    res = run_bass_kernel_spmd(nc, in_maps, list(range(NCORES)), **spmd_kwargs)
    out = np.concatenate([res.results[k]["out_s"] for k in range(NCORES)], axis=1)
    return out.astype(np.float32), res


def kernel(**inputs):
    out, _ = run(inputs)
    return out
